# revision 1
# baseline (speedup 1.0000x reference)
"""Trainium2 Bass kernel for CannyExtractor (NMS-suppressed canny magnitude).

Contract: kernel(x) takes the FULL input x [16,3,512,512] f32 and returns the
FULL output [16,3,512,512] f32. Internally shards the batch over 8 NeuronCores
(2 images per core), runs one SPMD Bass program, and reassembles.

Pipeline per image:
  gray (DVE+POOL scalar_tensor_tensor) -> horizontal 5-tap gaussian on the
  SINGLE gray plane (fp32, DVE+POOL; the gaussian commutes with the vertical
  composite so running it first halves the fp32 horizontal work) -> vertical
  composite convs on PE (banded fp32 matmuls + corner matmul for inter-block
  halos) -> 3-tap horizontal gradients (fp32) -> squares on ACT -> fp16
  compare field: NMS axis selection + suppression entirely in fp16 (DVE TT
  ops run 2x on 2-byte dtypes), row-shifted planes via exact fp16 PE
  permutation matmuls. Output = sqrt(s*keep) clipped to [0,1]; the sqrt runs
  on ACT from the keep-masked fp16 squared magnitude, so suppressed pixels
  are exactly 0 and kept pixels carry fp32-grade values (fp16 quantization
  of the output is ~2e-4 relative).

All elementwise ops are emitted flat over the whole [128, 4*516] plane
(block-boundary garbage lands in guard columns that are either rewritten or
never read). Tile storage is explicitly recycled: each pool tag hosts a
chain of logically-distinct planes whose lifetimes do not overlap.
"""
import sys
import numpy as np

sys.path.insert(0, "/opt/trn_rl_repo")

H = W = 512
NT = 4            # 128-row blocks per image
P = 128
PAD = 2
WP = W + 2 * PAD  # padded plane width (516)
L = NT * WP       # flat free length (2064)
LV = L - 4        # flat op length (2060): covers every block's 512 cols
NI = 2            # images per core
NCORES = 8

GRAY = np.array([0.299, 0.587, 0.114], np.float32)
SQT2 = np.float32(np.sqrt(2.0) - 1.0)        # tan(22.5 deg)


def _gauss5():
    ax = np.arange(5, dtype=np.float32) - 2.0
    g1 = np.exp(-0.5 * ax * ax).astype(np.float32)
    return (g1 / g1.sum()).astype(np.float32)


def _vert_matrix(kind):
    """512x512 M[o,i]: vertical composite (3-tap sobel part o replicate-pad o
    gaussian o reflect-pad), float64."""
    g1 = _gauss5()
    I = np.eye(H, dtype=np.float64)
    X = np.pad(I, ((2, 2), (0, 0)), mode="reflect")
    B = np.zeros((H, H))
    for k in range(5):
        B += g1[k] * X[k:k + H]
    Y = np.pad(B, ((1, 1), (0, 0)), mode="edge")
    taps = [1.0, 2.0, 1.0] if kind == "smooth" else [-1.0, 0.0, 1.0]
    M = np.zeros((H, H))
    for k in range(3):
        if taps[k] != 0.0:
            M += taps[k] * Y[k:k + H]
    return M


def _build_consts():
    Ms = (_vert_matrix("smooth") * float(GRAY[2])).astype(np.float32)
    Md = (_vert_matrix("diff") * float(GRAY[2])).astype(np.float32)
    vs = np.zeros((P, NT, P), np.float32)
    vd = np.zeros((P, NT, P), np.float32)
    for t in range(NT):
        vs[:, t, :] = Ms[128 * t:128 * (t + 1), 128 * t:128 * (t + 1)].T
        vd[:, t, :] = Md[128 * t:128 * (t + 1), 128 * t:128 * (t + 1)].T
    vcor = np.zeros((36, 2, 18), np.float32)
    for b in range(3):
        in_rows = [128 * b + 122 + k for k in range(12)]
        out_rows = [128 * b + 125, 128 * b + 126, 128 * b + 127,
                    128 * (b + 1), 128 * (b + 1) + 1, 128 * (b + 1) + 2]
        for k, ir in enumerate(in_rows):
            for m, orr in enumerate(out_rows):
                vcor[12 * b + k, 0, 6 * b + m] = Ms[orr, ir]
                vcor[12 * b + k, 1, 6 * b + m] = Md[orr, ir]
    # fp16 row-shift matrices: up[k,m]=1 iff k=m+1 (U[m]=s[m+1]); dn: k=m-1
    shm = np.zeros((P, 2, P), np.float16)
    for m in range(P - 1):
        shm[m + 1, 0, m] = 1.0
    for m in range(1, P):
        shm[m - 1, 1, m] = 1.0
    return {"vs": vs, "vd": vd, "vcor": vcor, "shm16": shm}


_CACHE = {}


def _emit_image(nc, tc, pools, tens, img):
    """Generator: yields between pipeline stages so the caller can interleave
    the two images' stages for cross-image engine overlap."""
    import concourse.mybir as mybir
    AL = mybir.AluOpType
    AF = mybir.ActivationFunctionType
    F32 = mybir.dt.float32
    F16 = mybir.dt.float16

    pwork, pw16, psmall, ppsum = pools
    (xdram, ydram, c_vs, c_vd, c_vcor, c_shm16, zrow16) = tens

    g1 = _gauss5()
    C0, C1, C2 = float(g1[2]), float(g1[1]), float(g1[0])
    R01 = float(np.float32(GRAY[0] / GRAY[1]))
    R12 = float(np.float32(GRAY[1] / GRAY[2]))

    INT = slice(PAD, PAD + W)       # interior columns of a 516-wide block

    def wt(tag):
        t = pwork.tile([P, NT, WP], F32, tag=tag, name=tag)
        return t, t[:].rearrange("p t w -> p (t w)")

    def wt16(tag):
        t = pw16.tile([P, NT, WP], F16, tag=tag, name=tag)
        return t, t[:].rearrange("p t w -> p (t w)")

    def wtu16(tag):
        # uint16 masks: CopyPredicated requires an integer mask dtype, and
        # 2-byte outputs keep the producing compare ops in 2x DVE mode
        t = pw16.tile([P, NT, WP], mybir.dt.uint16, tag=tag, name=tag)
        return t, t[:].rearrange("p t w -> p (t w)")

    # ---- load input channels ----
    x0, x0f = wt("X0")
    x1, x1f = wt("X1")
    x2, x2f = wt("X2")
    for c, t in enumerate((x0, x1, x2)):
        for b in range(NT):
            nc.sync.dma_start(
                t[:, b, INT],
                xdram[img, c].rearrange("(t p) w -> p t w", p=P)[:, b, :])

    # ---- grayscale (x0.114 folded into the vertical matrices) ----
    # gtmp -> X0 (in place), g -> X0 (in place); X1, X2 free afterwards
    nc.vector.scalar_tensor_tensor(x0f[:, PAD:L], x0f[:, PAD:L],
                                   R01, x1f[:, PAD:L], AL.mult, AL.add)
    nc.vector.scalar_tensor_tensor(x0f[:, PAD:L], x0f[:, PAD:L],
                                   R12, x2f[:, PAD:L], AL.mult, AL.add)
    g, gf = x0, x0f
    # reflect guard columns for the horizontal gaussian
    nc.scalar.copy(g[:, :, 1:2], g[:, :, 3:4])
    nc.scalar.copy(g[:, :, 0:1], g[:, :, 4:5])
    nc.scalar.copy(g[:, :, WP - 2:WP - 1], g[:, :, WP - 4:WP - 3])
    nc.scalar.copy(g[:, :, WP - 1:WP], g[:, :, WP - 5:WP - 4])
    yield

    # ---- horizontal gaussian on the single gray plane (fp32) ----
    # out col c holds the value for interior col c+2 (base-0 storage)
    a1, a1f = x1, x1f     # reuse: xc1 is dead after the gray stage
    a2, a2f = x2, x2f     # reuse: xc2 is dead after the gray stage
    q1, q1f = wt("Q")
    gh, ghf = wt("GH")
    nc.gpsimd.tensor_tensor(a1f[:, 0:LV], gf[:, 1:1 + LV], gf[:, 3:3 + LV], AL.add)
    nc.gpsimd.tensor_tensor(a2f[:, 0:LV], gf[:, 0:LV], gf[:, 4:4 + LV], AL.add)
    nc.vector.scalar_tensor_tensor(q1f[:, 0:LV], a2f[:, 0:LV], C2 / C1,
                                   a1f[:, 0:LV], AL.mult, AL.add)
    nc.vector.scalar_tensor_tensor(ghf[:, 2:2 + LV], q1f[:, 0:LV], C1 / C0,
                                   gf[:, 2:2 + LV], AL.mult, AL.add)
    yield

    # ---- vertical composite convs on PE (fp32) ----
    u1, u1f = wt("U1")
    u2, u2f = wt("U2")
    for t in range(NT):
        for (cm, u) in ((c_vs, u1), (c_vd, u2)):
            psb = ppsum.tile([P, W], F32, tag="ps", name="ps")
            nc.tensor.matmul(psb[:], cm[:, t, :], gh[:, t, INT], start=True, stop=True)
            nc.scalar.copy(u[:, t, INT], psb[:])
    cs = psmall.tile([36, W], F32, tag="cs", name="cs")
    for b in range(3):
        nc.sync.dma_start(cs[12 * b:12 * b + 6, :], gh[122:128, b, INT])
        nc.sync.dma_start(cs[12 * b + 6:12 * b + 12, :], gh[0:6, b + 1, INT])
    for ci, u in ((0, u1), (1, u2)):
        cps = ppsum.tile([18, W], F32, tag="ps", name="ps")
        nc.tensor.matmul(cps[:], c_vcor[:, ci, :], cs[:], start=True, stop=True)
        co = psmall.tile([18, W], F32, tag="co", name="co")
        nc.scalar.copy(co[:], cps[:])
        for b in range(3):
            nc.sync.dma_start(u[125:128, b, INT], co[6 * b:6 * b + 3, :])
            nc.sync.dma_start(u[0:3, b + 1, INT], co[6 * b + 3:6 * b + 6, :])
    # replicate guard columns (one col each side) for the 3-tap stage
    for u in (u1, u2):
        nc.scalar.copy(u[:, :, 1:2], u[:, :, 2:3])
        nc.scalar.copy(u[:, :, WP - 2:WP - 1], u[:, :, WP - 3:WP - 2])
    yield

    # ---- 3-tap horizontal gradients (fp32); base-0 storage ----
    gx, gxf = x1, x1f     # reuse: a1 is dead after q1
    ay, ayf = x2, x2f     # reuse: a2 is dead after q1
    gy, gyf = q1, q1f     # reuse: q1 is dead after gh
    nc.vector.tensor_tensor(gxf[:, 0:LV], u1f[:, 3:3 + LV], u1f[:, 1:1 + LV],
                            AL.subtract)
    nc.gpsimd.tensor_tensor(ayf[:, 0:LV], u2f[:, 1:1 + LV], u2f[:, 3:3 + LV], AL.add)
    nc.vector.scalar_tensor_tensor(gyf[:, 0:LV], u2f[:, 2:2 + LV], 2.0,
                                   ayf[:, 0:LV], AL.mult, AL.add)
    yield

    # ---- squares (ACT), s32 (POOL), fp16 compare-field inputs ----
    sqx, sqxf = x0, x0f   # reuse: g is dead after gh
    sqy, sqyf = gh, ghf   # reuse: gh is dead after the vertical stage
    s32, s32f = x2, x2f   # reuse: ay is dead after gy
    y2, y2f = wt16("Y2")
    t2x, t2xf = wt16("T2X")
    gx16, gx16f = wt16("GX")
    gy16, gy16f = wt16("GY")
    s16, s16f = wt16("S16")
    nc.scalar.activation(sqxf[:, 0:LV], gxf[:, 0:LV], AF.Square, 0.0, C0)
    nc.scalar.activation(sqyf[:, 0:LV], gyf[:, 0:LV], AF.Square, 0.0, C0)
    nc.gpsimd.tensor_tensor(s32f[:, 0:LV], sqxf[:, 0:LV], sqyf[:, 0:LV], AL.add)
    nc.scalar.activation(y2f[:, 0:LV], gyf[:, 0:LV], AF.Square, 0.0, C0)
    nc.scalar.activation(t2xf[:, 0:LV], gxf[:, 0:LV], AF.Square, 0.0,
                         C0 * float(SQT2))
    nc.scalar.copy(gx16f[:, 0:LV], gxf[:, 0:LV])
    nc.scalar.copy(gy16f[:, 0:LV], gyf[:, 0:LV])
    # s16 at interior base 2 with zeroed guard cols for the NMS shifts
    nc.scalar.copy(s16f[:, 2:2 + LV], s32f[:, 0:LV])
    nc.gpsimd.memset(s16[:, :, 0:PAD], 0.0)
    nc.gpsimd.memset(s16[:, :, WP - PAD:WP], 0.0)
    yield

    # ---- row-shifted planes U16[r]=s16[r+1], D16[r]=s16[r-1] (fp16 PE) ----
    U16, U16f = wt16("U16")
    D16, D16f = wt16("D16")
    for t in range(NT):
        for (ci, pl) in ((0, U16), (1, D16)):
            psb = ppsum.tile([P, W], F32, tag="ps", name="ps")
            nc.tensor.matmul(psb[:], c_shm16[:, ci, :], s16[:, t, INT],
                             start=True, stop=True)
            nc.scalar.copy(pl[:, t, INT], psb[:])
    for pl in (U16, D16):
        nc.gpsimd.memset(pl[:, :, 0:PAD], 0.0)
        nc.gpsimd.memset(pl[:, :, WP - PAD:WP], 0.0)
    for t in range(NT - 1):
        nc.sync.dma_start(U16[127:128, t, INT], s16[0:1, t + 1, INT])
    nc.sync.dma_start(U16[127:128, NT - 1, INT], zrow16[0:1, :])
    for t in range(1, NT):
        nc.sync.dma_start(D16[0:1, t, INT], s16[127:128, t - 1, INT])
    nc.gpsimd.memset(D16[0:1, 0, INT], 0.0)
    yield

    # ---- fp16 NMS: masks, pair maxes, select ----
    ch, chf = wtu16("CH")
    cv, cvf = wtu16("CV")
    md1, md1f = wtu16("MD")
    # ch = (T2*gx^2 >= gy^2)
    nc.vector.tensor_tensor(chf[:, 0:LV], t2xf[:, 0:LV], y2f[:, 0:LV], AL.is_ge)
    # xv = gx^2 / T2, reusing the t2x slot; cv = (gy^2 > gx^2/T2)
    nc.scalar.activation(t2xf[:, 0:LV], gxf[:, 0:LV], AF.Square, 0.0,
                         C0 / float(SQT2))
    nc.vector.tensor_tensor(cvf[:, 0:LV], y2f[:, 0:LV], t2xf[:, 0:LV], AL.is_gt)
    # pxy in place in the GX slot, then md1 = (pxy > 0) as a uint16 mask
    nc.vector.tensor_tensor(gx16f[:, 0:LV], gx16f[:, 0:LV], gy16f[:, 0:LV],
                            AL.mult)
    nc.vector.tensor_scalar(md1f[:, 0:LV], gx16f[:, 0:LV], 0.0, None, AL.is_gt)
    # pair maxes: mh -> T2X slot, mv -> GY slot, m1 -> Y2 slot, m3/sel -> D16
    nc.vector.tensor_tensor(t2xf[:, 0:LV], s16f[:, 1:1 + LV], s16f[:, 3:3 + LV],
                            AL.max)
    mhf = t2xf
    nc.vector.tensor_tensor(gy16f[:, 0:LV], U16f[:, 2:2 + LV], D16f[:, 2:2 + LV],
                            AL.max)
    mvf = gy16f
    nc.vector.tensor_tensor(y2f[:, 0:LV], U16f[:, 3:3 + LV], D16f[:, 1:1 + LV],
                            AL.max)
    m1f = y2f
    # m3 written into D16 (reads are forward-shifted: safe in-place)
    nc.vector.tensor_tensor(D16f[:, 0:LV], U16f[:, 1:1 + LV], D16f[:, 3:3 + LV],
                            AL.max)
    self_ = D16f
    nc.vector.copy_predicated(self_[:, 0:LV], md1f[:, 0:LV], m1f[:, 0:LV])
    nc.vector.copy_predicated(self_[:, 0:LV], cvf[:, 0:LV], mvf[:, 0:LV])
    nc.vector.copy_predicated(self_[:, 0:LV], chf[:, 0:LV], mhf[:, 0:LV])
    yield

    # ---- keep, magnitude, clip, store ----
    out32, out32f = u1, u1f   # reuse: u1 is dead after gx
    # keep -> Y2 slot (m1 consumed by the first copy_predicated above)
    nc.vector.tensor_tensor(y2f[:, 0:LV], s16f[:, 2:2 + LV], self_[:, 0:LV],
                            AL.is_gt)
    keepf = y2f
    # s16k = s16 * keep, in place on s16 (aligned)
    nc.vector.tensor_tensor(s16f[:, 2:2 + LV], s16f[:, 2:2 + LV], keepf[:, 0:LV],
                            AL.mult)
    # mag -> U16 slot; clip via tensor_scalar_min -> GX slot; widen -> U1 slot
    nc.scalar.activation(U16f[:, 0:LV], s16f[:, 2:2 + LV], AF.Sqrt, 0.0, 1.0)
    nc.vector.tensor_scalar_min(gx16f[:, 0:LV], U16f[:, 0:LV], 1.0)
    nc.scalar.copy(out32f[:, 0:LV], gx16f[:, 0:LV])
    for c in range(3):
        for b in range(NT):
            nc.sync.dma_start(
                ydram[img, c].rearrange("(t p) w -> p t w", p=P)[:, b, :],
                out32[:, b, 0:W])
    yield


def _build():
    import concourse.bacc as bacc
    import concourse.mybir as mybir
    from concourse import tile
    F32 = mybir.dt.float32
    F16 = mybir.dt.float16

    nc = bacc.Bacc("TRN2", target_bir_lowering=False, debug=False,
                   num_devices=NCORES)
    xdram = nc.declare_dram_parameter("xc", [NI, 3, H, W], F32, isOutput=False)
    c_vs_d = nc.declare_dram_parameter("vs", [P, NT, P], F32, isOutput=False)
    c_vd_d = nc.declare_dram_parameter("vd", [P, NT, P], F32, isOutput=False)
    c_vcor_d = nc.declare_dram_parameter("vcor", [36, 2, 18], F32, isOutput=False)
    c_shm_d = nc.declare_dram_parameter("shm16", [P, 2, P], F16, isOutput=False)
    ydram = nc.declare_dram_parameter("y", [NI, 3, H, W], F32, isOutput=True)

    with tile.TileContext(nc) as tc:
        with tc.tile_pool(name="pconst", bufs=1) as pconst, \
             tc.tile_pool(name="pwork", bufs=2) as pwork, \
             tc.tile_pool(name="pw16", bufs=2) as pw16, \
             tc.tile_pool(name="psmall", bufs=1) as psmall, \
             tc.tile_pool(name="ppsum", bufs=6, space="PSUM") as ppsum:
            c_vs = pconst.tile([P, NT, P], F32, tag="cvs")
            nc.sync.dma_start(c_vs[:], c_vs_d[:])
            c_vd = pconst.tile([P, NT, P], F32, tag="cvd")
            nc.sync.dma_start(c_vd[:], c_vd_d[:])
            c_vcor = pconst.tile([36, 2, 18], F32, tag="cvcor")
            nc.sync.dma_start(c_vcor[:], c_vcor_d[:])
            c_shm16 = pconst.tile([P, 2, P], F16, tag="cshm")
            nc.sync.dma_start(c_shm16[:], c_shm_d[:])
            zrow16 = pconst.tile([1, W], F16, tag="zr16")
            nc.gpsimd.memset(zrow16[:], 0.0)

            pools = (pwork, pw16, psmall, ppsum)
            tens = (xdram, ydram, c_vs, c_vd, c_vcor, c_shm16, zrow16)
            import os
            nrep = int(os.environ.get("KREPEAT", "1"))
            for rep in range(nrep):
                gens = [_emit_image(nc, tc, pools, tens, img) for img in range(NI)]
                done = [False] * NI
                while not all(done):
                    for i, gi in enumerate(gens):
                        if not done[i]:
                            try:
                                next(gi)
                            except StopIteration:
                                done[i] = True

    nc.compile()
    return nc


def _get_nc():
    if "nc" not in _CACHE:
        _CACHE["nc"] = _build()
        _CACHE["consts"] = _build_consts()
    return _CACHE["nc"], _CACHE["consts"]


def kernel(x):
    from concourse.bass_utils import run_bass_kernel_spmd
    x = np.ascontiguousarray(np.asarray(x), dtype=np.float32)
    assert x.shape == (16, 3, H, W), x.shape
    nc, consts = _get_nc()
    in_maps = []
    for c in range(NCORES):
        m = {"xc": x[NI * c:NI * (c + 1)]}
        m.update(consts)
        in_maps.append(m)
    res = run_bass_kernel_spmd(nc, in_maps, list(range(NCORES)))
    y = np.concatenate([res.results[c]["y"] for c in range(NCORES)], axis=0)
    return y.astype(np.float32)


if __name__ == "__main__":
    import golden
    rng = np.random.default_rng(0)
    x = rng.random((16, 3, H, W), dtype=np.float32)
    y = kernel(x)
    ref = golden.reference_np(x)
    d = y - ref
    print("L2 rel:", np.linalg.norm(d) / np.linalg.norm(ref))
    print("absmax:", np.abs(d).max(), " bigpix:", (np.abs(d) > 1e-3).sum())



# revision 9
# speedup vs baseline: 1.0185x; 1.0185x over previous
"""Trainium2 Bass kernel for CannyExtractor (NMS-suppressed canny magnitude).

Contract: kernel(x) takes FULL input x [16,3,512,512] f32, returns FULL output
[16,3,512,512] f32. Internally: batch sharded over 8 NeuronCores (2 images
per core), one SPMD Bass program, device emits the fp16 single-channel
suppressed magnitude; host widens to f32 and replicates the 3 identical
channels (reference output is channel-replicated).

Pipeline per image (fp32 until squares — fp16 compare field; the precision
split is forced: quantizing anything upstream of gx/gy to fp16 pushes L2
rel-err past the 2e-2 gate due to cancellation in the derivative taps):
  gray (DVE STT fp32) -> horizontal 5-tap gaussian (POOL adds + DVE STTs,
  fp32) -> vertical composite convs on PE (banded fp32 matmuls + corner
  matmul for inter-block halos, direct PSUM->SBUF flat relays on ACT) ->
  3-tap horizontal gradients (DVE/POOL fp32) -> squares on ACT (fp32) ->
  s16 fp16 compare field; NMS masks as strict-only compares (is_lt/is_gt;
  is_ge measured 3.2x slower than is_gt on DVE) -> row-shifted planes via
  fp16 PE matmuls -> pair maxes + copy_predicated select chain -> keep,
  sqrt (ACT, +eps bias), clip, apply -> fp16 out, one DMA per image.
"""
import sys
import numpy as np

sys.path.insert(0, "/opt/trn_rl_repo")

H = W = 512
NT = 4            # 128-row blocks per image
P = 128
PAD = 2
WP = W + 2 * PAD  # padded plane width (516)
L = NT * WP       # flat free length (2064)
LV = L - 4        # flat op length (2060)
INT = slice(PAD, PAD + W)
NI = 2            # images per core
NCORES = 8

GRAY = np.array([0.299, 0.587, 0.114], np.float32)
SQT2 = np.float32(np.sqrt(2.0) - 1.0)        # tan(22.5 deg)
T2 = float(np.float32(SQT2 * SQT2))          # tan^2(22.5)
TH2 = float(np.float32(1.0 / (SQT2 * SQT2)))  # tan^2(67.5)


def _gauss5():
    ax = np.arange(5, dtype=np.float32) - 2.0
    g1 = np.exp(-0.5 * ax * ax).astype(np.float32)
    return (g1 / g1.sum()).astype(np.float32)


def _vert_matrix(kind):
    g1 = _gauss5()
    I = np.eye(H, dtype=np.float64)
    X = np.pad(I, ((2, 2), (0, 0)), mode="reflect")
    B = np.zeros((H, H))
    for k in range(5):
        B += g1[k] * X[k:k + H]
    Y = np.pad(B, ((1, 1), (0, 0)), mode="edge")
    taps = [1.0, 2.0, 1.0] if kind == "smooth" else [-1.0, 0.0, 1.0]
    M = np.zeros((H, H))
    for k in range(3):
        if taps[k] != 0.0:
            M += taps[k] * Y[k:k + H]
    return M


def _build_consts():
    Ms = (_vert_matrix("smooth") * float(GRAY[2])).astype(np.float32)
    Md = (_vert_matrix("diff") * float(GRAY[2])).astype(np.float32)
    vs = np.zeros((P, NT, P), np.float32)
    vd = np.zeros((P, NT, P), np.float32)
    for t in range(NT):
        vs[:, t, :] = Ms[128 * t:128 * (t + 1), 128 * t:128 * (t + 1)].T
        vd[:, t, :] = Md[128 * t:128 * (t + 1), 128 * t:128 * (t + 1)].T
    # halo matrices: vh[j, ud, sd, t, o] — 6 input rows j from the block
    # above (ud=0: rows 128t-6+j) or below (ud=1: rows 128(t+1)+j) block t,
    # mapped into block t's 128 output rows o; sd: 0=smooth, 1=diff.
    vh = np.zeros((6, 2, 2, NT, P), np.float32)
    for t in range(NT):
        for j in range(6):
            if t >= 1:
                ir = 128 * t - 6 + j
                for o in range(P):
                    vh[j, 0, 0, t, o] = Ms[128 * t + o, ir]
                    vh[j, 0, 1, t, o] = Md[128 * t + o, ir]
            if t < NT - 1:
                ir = 128 * (t + 1) + j
                for o in range(P):
                    vh[j, 1, 0, t, o] = Ms[128 * t + o, ir]
                    vh[j, 1, 1, t, o] = Md[128 * t + o, ir]
    # fp16 row-shift matrices: up[k,m]=1 iff k=m+1 (U[m]=s[m+1]); dn: k=m-1
    shm = np.zeros((P, 2, P), np.float16)
    for m in range(P - 1):
        shm[m + 1, 0, m] = 1.0
    for m in range(1, P):
        shm[m - 1, 1, m] = 1.0
    return {"vs": vs, "vd": vd, "vh": vh, "shm16": shm}


_CACHE = {}


def _emit_image(nc, tc, pools, tens, img):
    """Generator: yields between stages so the caller interleaves 2 images."""
    import concourse.mybir as mybir
    AL = mybir.AluOpType
    AF = mybir.ActivationFunctionType
    F32 = mybir.dt.float32
    F16 = mybir.dt.float16
    U16 = mybir.dt.uint16

    pwork, pw16, psmall, (ppsumv, ppsumc) = pools
    (xdram, ydram, c_vs, c_vd, c_vh, c_shm16, zrow16) = tens

    g1 = _gauss5()
    C0, C1, C2 = float(g1[2]), float(g1[1]), float(g1[0])
    R01 = float(np.float32(GRAY[0] / GRAY[1]))
    R12 = float(np.float32(GRAY[1] / GRAY[2]))

    def wt(tag):
        t = pwork.tile([P, NT, WP], F32, tag=tag, name=tag)
        return t, t[:].rearrange("p t w -> p (t w)")

    def wt16(tag, dt=F16):
        t = pw16.tile([P, NT, WP], dt, tag=tag, name=tag)
        return t, t[:].rearrange("p t w -> p (t w)")

    # ---- load input channels (per half: earlier pipeline start) ----
    xa, xaf = wt("A")
    xb, xbf = wt("B")
    xc, xcf = wt("C")
    for h in range(2):
        for c, t in enumerate((xa, xb, xc)):
            nc.sync.dma_start(
                t[:, 2 * h:2 * h + 2, INT],
                xdram[img, c].rearrange("(t p) w -> p t w", p=P)[:, 2 * h:2 * h + 2, :])
    yield

    # ---- grayscale (fp32, per half; scale 1/GRAY[2] folded into mats) ----
    HL = 2 * WP           # flat length of a half
    HV = HL - 4
    def half(f, h, lo, ln):
        return f[:, 2 * h * WP + lo: 2 * h * WP + lo + ln]
    g, gf = xc, xcf          # gray lands in xc, base-2
    for h in range(2):
        nc.vector.scalar_tensor_tensor(half(xbf, h, PAD, HL - PAD),
                                       half(xaf, h, PAD, HL - PAD), R01,
                                       half(xbf, h, PAD, HL - PAD),
                                       AL.mult, AL.add)
        nc.vector.scalar_tensor_tensor(half(xcf, h, PAD, HL - PAD),
                                       half(xbf, h, PAD, HL - PAD), R12,
                                       half(xcf, h, PAD, HL - PAD),
                                       AL.mult, AL.add)
        # reflect guard cols for the horizontal gaussian
        sl = slice(2 * h, 2 * h + 2)
        nc.scalar.copy(g[:, sl, 1:2], g[:, sl, 3:4])
        nc.scalar.copy(g[:, sl, 0:1], g[:, sl, 4:5])
        nc.scalar.copy(g[:, sl, WP - 2:WP - 1], g[:, sl, WP - 4:WP - 3])
        nc.scalar.copy(g[:, sl, WP - 1:WP], g[:, sl, WP - 5:WP - 4])
        yield

    # ---- horizontal 5-tap gaussian (fp32, per half), gh base-2 = blur/C0 --
    a1, a1f = xa, xaf        # xa dead after gray STT1
    a2, a2f = xb, xbf        # xb dead after gray STT2
    gh, ghf = wt("D")
    for h in range(2):
        nc.gpsimd.tensor_tensor(half(a1f, h, 0, HV), half(gf, h, 1, HV),
                                half(gf, h, 3, HV), AL.add)
        nc.gpsimd.tensor_tensor(half(a2f, h, 0, HV), half(gf, h, 0, HV),
                                half(gf, h, 4, HV), AL.add)
        nc.vector.scalar_tensor_tensor(half(a1f, h, 0, HV), half(a2f, h, 0, HV),
                                       C2 / C1, half(a1f, h, 0, HV),
                                       AL.mult, AL.add)
        nc.vector.scalar_tensor_tensor(half(ghf, h, 2, HV), half(a1f, h, 0, HV),
                                       C1 / C0, half(gf, h, 2, HV),
                                       AL.mult, AL.add)
        yield

    # ---- vertical composite on PE (fp32 banded matmul + halo matmuls) ----
    u1, u1f = xa, xaf        # a1 dead after gh
    u2, u2f = xb, xbf        # a2 dead after gh
    # up-halo rows must sit at partition base 0 for the PE moving operand
    halo_u = psmall.tile([6, NT - 1, W], F32, tag="hu", name="hu")
    nc.sync.dma_start(halo_u[:, 0:2, :], gh[122:128, 0:2, INT])
    nc.sync.dma_start(halo_u[:, 2:3, :], gh[122:128, 2:3, INT])
    for (sd, cm, u) in ((0, c_vs, u1), (1, c_vd, u2)):
        for h in range(2):
            ps = ppsumv.tile([P, 2, W], F32, tag="psv", name="psv")
            for k in range(2):
                t = 2 * h + k
                nc.tensor.matmul(ps[:, k, :], cm[:, t, :], gh[:, t, INT],
                                 start=True, stop=False)
                if t >= 1:
                    nc.tensor.matmul(ps[:, k, :], c_vh[:, 0, sd, t, :],
                                     halo_u[:, t - 1, :],
                                     start=False, stop=(t == NT - 1))
                if t < NT - 1:
                    nc.tensor.matmul(ps[:, k, :], c_vh[:, 1, sd, t, :],
                                     gh[0:6, t + 1, INT],
                                     start=False, stop=True)
            nc.scalar.copy(u[:, 2 * h:2 * h + 2, INT], ps[:])
    # replicate guard cols (one col each side) for the 3-tap stage
    for u in (u1, u2):
        nc.scalar.copy(u[:, :, 1:2], u[:, :, 2:3])
        nc.scalar.copy(u[:, :, WP - 2:WP - 1], u[:, :, WP - 3:WP - 2])
    yield

    # ---- 3-tap horizontal gradients (fp32, base-0) ----
    gx, gxf = gh, ghf        # gh dead after vertical+corner
    ay, ayf = wt("E")
    nc.vector.tensor_tensor(gxf[:, 0:LV], u1f[:, 3:3 + LV], u1f[:, 1:1 + LV],
                            AL.subtract)
    nc.gpsimd.tensor_tensor(ayf[:, 0:LV], u2f[:, 1:1 + LV], u2f[:, 3:3 + LV],
                            AL.add)
    nc.vector.scalar_tensor_tensor(ayf[:, 0:LV], u2f[:, 2:2 + LV], 2.0,
                                   ayf[:, 0:LV], AL.mult, AL.add)
    gy, gyf = ay, ayf
    yield

    # ---- squares (ACT fp32), s16 fp16 compare field, masks ----
    sx, sxf = u1, u1f        # u1 dead after gx
    sy, syf = u2, u2f        # u2 dead after gy
    nc.scalar.activation(sxf[:, 0:LV], gxf[:, 0:LV], AF.Square, 0.0, C0)
    nc.scalar.activation(syf[:, 0:LV], gyf[:, 0:LV], AF.Square, 0.0, C0)
    s16, s16f = wt16("S16")
    nc.vector.tensor_tensor(s16f[:, 2:2 + LV], sxf[:, 0:LV], syf[:, 0:LV],
                            AL.add)
    nc.gpsimd.memset(s16[:, :, 0:PAD], 0.0)
    nc.gpsimd.memset(s16[:, :, WP - PAD:WP], 0.0)
    # masks: notch = (T2*sx < sy), cv = (TH2*sx < sy)  [strict compares only]
    notch, notchf = wt16("NCH", U16)
    cvm, cvmf = wt16("CV", U16)
    nc.vector.scalar_tensor_tensor(notchf[:, 0:LV], sxf[:, 0:LV], T2,
                                   syf[:, 0:LV], AL.mult, AL.is_lt)
    nc.vector.scalar_tensor_tensor(cvmf[:, 0:LV], sxf[:, 0:LV], TH2,
                                   syf[:, 0:LV], AL.mult, AL.is_lt)
    # md1 = (gx*gy > 0): fp16 product then 4x-mode fp16 TS compare
    pxy, pxyf = wt16("PXY")
    nc.vector.tensor_tensor(pxyf[:, 0:LV], gxf[:, 0:LV], gyf[:, 0:LV],
                            AL.mult)
    md1, md1f = wt16("MD", U16)
    nc.vector.tensor_scalar(md1f[:, 0:LV], pxyf[:, 0:LV], 0.0, None, AL.is_gt)
    yield

    # ---- row-shifted planes U16[r]=s16[r+1], D16[r]=s16[r-1] (fp16 PE) ----
    U16t, U16f = wt16("U16")
    D16t, D16f = wt16("D16")
    for (ci, pl) in ((0, U16t), (1, D16t)):
        for h in range(2):
            ps = ppsumv.tile([P, 2, W], F32, tag="psv", name="psv")
            for k in range(2):
                t = 2 * h + k
                nc.tensor.matmul(ps[:, k, :], c_shm16[:, ci, :],
                                 s16[:, t, INT], start=True, stop=True)
            nc.scalar.copy(pl[:, 2 * h:2 * h + 2, INT], ps[:])
    for pl in (U16t, D16t):
        nc.gpsimd.memset(pl[:, :, 0:PAD], 0.0)
        nc.gpsimd.memset(pl[:, :, WP - PAD:WP], 0.0)
    # inter-block boundary rows (one consolidated DMA each) + edge zeros
    nc.sync.dma_start(
        U16t[127:128, 0:NT - 1, INT],
        s16[0:1, 1:NT, INT])
    nc.sync.dma_start(U16t[127:128, NT - 1, INT], zrow16[0:1, :])
    nc.sync.dma_start(
        D16t[0:1, 1:NT, INT],
        s16[127:128, 0:NT - 1, INT])
    nc.gpsimd.memset(D16t[0:1, 0, INT], 0.0)
    yield

    # ---- pair maxes + select chain (fp16) ----
    selx, selxf = wt16("SELX")   # base m3, becomes diag/vert select
    m1t, m1tf = wt16("M1")
    mvt, mvtf = wt16("MV")
    sel, self_ = wt16("SEL")     # base mh, becomes final selection
    nc.vector.tensor_tensor(selxf[:, 0:LV], U16f[:, 1:1 + LV],
                            D16f[:, 3:3 + LV], AL.max)
    nc.vector.tensor_tensor(m1tf[:, 0:LV], U16f[:, 3:3 + LV],
                            D16f[:, 1:1 + LV], AL.max)
    nc.vector.tensor_tensor(mvtf[:, 0:LV], U16f[:, 2:2 + LV],
                            D16f[:, 2:2 + LV], AL.max)
    nc.vector.tensor_tensor(self_[:, 0:LV], s16f[:, 1:1 + LV],
                            s16f[:, 3:3 + LV], AL.max)
    nc.vector.copy_predicated(selxf[:, 0:LV], md1f[:, 0:LV], m1tf[:, 0:LV])
    nc.vector.copy_predicated(selxf[:, 0:LV], cvmf[:, 0:LV], mvtf[:, 0:LV])
    nc.vector.copy_predicated(self_[:, 0:LV], notchf[:, 0:LV], selxf[:, 0:LV])
    yield

    # ---- keep, magnitude, clip, store (fp16 out) ----
    keep, keepf = m1t, m1tf      # m1 consumed by first cp
    nc.vector.tensor_tensor(keepf[:, 0:LV], s16f[:, 2:2 + LV],
                            self_[:, 0:LV], AL.is_gt)
    mag, magf = mvt, mvtf        # mv consumed by second cp
    nc.scalar.activation(magf[:, 0:LV], s16f[:, 2:2 + LV], AF.Sqrt,
                         0.0, 1.0)
    nc.vector.tensor_scalar(magf[:, 0:LV], magf[:, 0:LV], 1.0, None, AL.min)
    out16, out16f = selx, selxf  # selx consumed by final cp
    nc.vector.tensor_tensor(out16f[:, 0:LV], magf[:, 0:LV], keepf[:, 0:LV],
                            AL.mult)
    nc.sync.dma_start(
        ydram[img].rearrange("(t p) w -> p t w", p=P),
        out16[:, :, 0:W])
    yield


def _build():
    import concourse.bacc as bacc
    import concourse.mybir as mybir
    from concourse import tile
    F32 = mybir.dt.float32
    F16 = mybir.dt.float16

    nc = bacc.Bacc("TRN2", target_bir_lowering=False, debug=False,
                   num_devices=NCORES)
    xdram = nc.declare_dram_parameter("xc", [NI, 3, H, W], F32, isOutput=False)
    c_vs_d = nc.declare_dram_parameter("vs", [P, NT, P], F32, isOutput=False)
    c_vd_d = nc.declare_dram_parameter("vd", [P, NT, P], F32, isOutput=False)
    c_vh_d = nc.declare_dram_parameter("vh", [6, 2, 2, NT, P], F32,
                                       isOutput=False)
    c_shm_d = nc.declare_dram_parameter("shm16", [P, 2, P], F16,
                                        isOutput=False)
    ydram = nc.declare_dram_parameter("y", [NI, H, W], F16, isOutput=True)

    with tile.TileContext(nc) as tc:
        with tc.tile_pool(name="pconst", bufs=1) as pconst, \
             tc.tile_pool(name="pwork", bufs=2) as pwork, \
             tc.tile_pool(name="pw16", bufs=2) as pw16, \
             tc.tile_pool(name="psmall", bufs=2) as psmall, \
             tc.tile_pool(name="ppsumv", bufs=3, space="PSUM") as ppsumv, \
             tc.tile_pool(name="ppsumc", bufs=2, space="PSUM") as ppsumc:
            c_vs = pconst.tile([P, NT, P], F32, tag="cvs")
            nc.sync.dma_start(c_vs[:], c_vs_d[:])
            c_vd = pconst.tile([P, NT, P], F32, tag="cvd")
            nc.sync.dma_start(c_vd[:], c_vd_d[:])
            c_vh = pconst.tile([6, 2, 2, NT, P], F32, tag="cvh")
            nc.sync.dma_start(c_vh[:], c_vh_d[:])
            c_shm16 = pconst.tile([P, 2, P], F16, tag="cshm")
            nc.sync.dma_start(c_shm16[:], c_shm_d[:])
            zrow16 = pconst.tile([1, W], F16, tag="zr16")
            nc.gpsimd.memset(zrow16[:], 0.0)

            pools = (pwork, pw16, psmall, (ppsumv, ppsumc))
            tens = (xdram, ydram, c_vs, c_vd, c_vh, c_shm16, zrow16)
            import os
            nrep = int(os.environ.get("KREPEAT", "1"))
            for rep in range(nrep):
                gens = [_emit_image(nc, tc, pools, tens, img)
                        for img in range(NI)]
                done = [False] * NI
                while not all(done):
                    for i, gi in enumerate(gens):
                        if not done[i]:
                            try:
                                next(gi)
                            except StopIteration:
                                done[i] = True

    nc.compile()
    return nc


def _get_nc():
    if "nc" not in _CACHE:
        _CACHE["nc"] = _build()
        _CACHE["consts"] = _build_consts()
    return _CACHE["nc"], _CACHE["consts"]


def kernel(x):
    from concourse.bass_utils import run_bass_kernel_spmd
    x = np.ascontiguousarray(np.asarray(x), dtype=np.float32)
    assert x.shape == (16, 3, H, W), x.shape
    nc, consts = _get_nc()
    in_maps = []
    for c in range(NCORES):
        m = {"xc": x[NI * c:NI * (c + 1)]}
        m.update(consts)
        in_maps.append(m)
    res = run_bass_kernel_spmd(nc, in_maps, list(range(NCORES)))
    y = np.concatenate([res.results[c]["y"] for c in range(NCORES)], axis=0)
    y = y.astype(np.float32)[:, None]          # widen fp16 -> f32, add C dim
    return np.repeat(y, 3, axis=1)             # replicate 3 identical channels


if __name__ == "__main__":
    import golden
    rng = np.random.default_rng(0)
    x = rng.random((16, 3, H, W), dtype=np.float32)
    y = kernel(x)
    ref = golden.reference_np(x)
    d = y - ref
    print("L2 rel:", np.linalg.norm(d) / np.linalg.norm(ref))
    print("absmax:", np.abs(d).max(), " bigpix:", (np.abs(d) > 1e-3).sum())


# revision 10
# speedup vs baseline: 1.2052x; 1.1833x over previous
"""Trainium2 Bass kernel for CannyExtractor (NMS-suppressed canny magnitude).

Contract: kernel(x) takes FULL input x [16,3,512,512] f32, returns FULL output
[16,3,512,512] f32. Internally: batch sharded over 8 NeuronCores (2 images
per core), one SPMD Bass program, device emits the fp16 single-channel
suppressed magnitude; host widens to f32 and replicates the 3 identical
channels (reference output is channel-replicated).

Pipeline per image (fp32 until squares — fp16 compare field; the precision
split is forced: quantizing anything upstream of gx/gy to fp16 pushes L2
rel-err past the 2e-2 gate due to cancellation in the derivative taps):
  gray (DVE STT fp32) -> horizontal 5-tap gaussian (POOL adds + DVE STTs,
  fp32) -> vertical composite convs on PE (banded fp32 matmuls + corner
  matmul for inter-block halos, direct PSUM->SBUF flat relays on ACT) ->
  3-tap horizontal gradients (DVE/POOL fp32) -> squares on ACT (fp32) ->
  s16 fp16 compare field; NMS masks as strict-only compares (is_lt/is_gt;
  is_ge measured 3.2x slower than is_gt on DVE) -> row-shifted planes via
  fp16 PE matmuls -> pair maxes + copy_predicated select chain -> keep,
  sqrt (ACT, +eps bias), clip, apply -> fp16 out, one DMA per image.
"""
import sys
import numpy as np

sys.path.insert(0, "/opt/trn_rl_repo")

H = W = 512
NT = 4            # 128-row blocks per image
P = 128
PAD = 2
WP = W + 2 * PAD  # padded plane width (516)
L = NT * WP       # flat free length (2064)
LV = L - 4        # flat op length (2060)
INT = slice(PAD, PAD + W)
NI = 2            # images per core
NCORES = 8

GRAY = np.array([0.299, 0.587, 0.114], np.float32)
SQT2 = np.float32(np.sqrt(2.0) - 1.0)        # tan(22.5 deg)
T2 = float(np.float32(SQT2 * SQT2))          # tan^2(22.5)
TH2 = float(np.float32(1.0 / (SQT2 * SQT2)))  # tan^2(67.5)


def _gauss5():
    ax = np.arange(5, dtype=np.float32) - 2.0
    g1 = np.exp(-0.5 * ax * ax).astype(np.float32)
    return (g1 / g1.sum()).astype(np.float32)


def _vert_matrix(kind):
    g1 = _gauss5()
    I = np.eye(H, dtype=np.float64)
    X = np.pad(I, ((2, 2), (0, 0)), mode="reflect")
    B = np.zeros((H, H))
    for k in range(5):
        B += g1[k] * X[k:k + H]
    Y = np.pad(B, ((1, 1), (0, 0)), mode="edge")
    taps = [1.0, 2.0, 1.0] if kind == "smooth" else [-1.0, 0.0, 1.0]
    M = np.zeros((H, H))
    for k in range(3):
        if taps[k] != 0.0:
            M += taps[k] * Y[k:k + H]
    return M


def _build_consts():
    Ms = (_vert_matrix("smooth") * float(GRAY[2])).astype(np.float32)
    Md = (_vert_matrix("diff") * float(GRAY[2])).astype(np.float32)
    vs = np.zeros((P, NT, P), np.float32)
    vd = np.zeros((P, NT, P), np.float32)
    for t in range(NT):
        vs[:, t, :] = Ms[128 * t:128 * (t + 1), 128 * t:128 * (t + 1)].T
        vd[:, t, :] = Md[128 * t:128 * (t + 1), 128 * t:128 * (t + 1)].T
    vcor = np.zeros((36, 2, 18), np.float32)
    for b in range(3):
        in_rows = [128 * b + 122 + k for k in range(12)]
        out_rows = [128 * b + 125, 128 * b + 126, 128 * b + 127,
                    128 * (b + 1), 128 * (b + 1) + 1, 128 * (b + 1) + 2]
        for k, ir in enumerate(in_rows):
            for m, orr in enumerate(out_rows):
                vcor[12 * b + k, 0, 6 * b + m] = Ms[orr, ir]
                vcor[12 * b + k, 1, 6 * b + m] = Md[orr, ir]
    # fp16 row-shift matrices: up[k,m]=1 iff k=m+1 (U[m]=s[m+1]); dn: k=m-1
    shm = np.zeros((P, 2, P), np.float16)
    for m in range(P - 1):
        shm[m + 1, 0, m] = 1.0
    for m in range(1, P):
        shm[m - 1, 1, m] = 1.0
    return {"vs": vs, "vd": vd, "vcor": vcor, "shm16": shm}


_CACHE = {}


def _emit_image(nc, tc, pools, tens, img):
    """Generator: yields between stages so the caller interleaves 2 images."""
    import concourse.mybir as mybir
    AL = mybir.AluOpType
    AF = mybir.ActivationFunctionType
    F32 = mybir.dt.float32
    F16 = mybir.dt.float16
    U16 = mybir.dt.uint16

    pwork, pw16, psmall, (ppsumv, ppsumc) = pools
    (xdram, ydram, c_vs, c_vd, c_vcor, c_shm16, zrow16) = tens

    g1 = _gauss5()
    C0, C1, C2 = float(g1[2]), float(g1[1]), float(g1[0])
    R01 = float(np.float32(GRAY[0] / GRAY[1]))
    R12 = float(np.float32(GRAY[1] / GRAY[2]))

    def wt(tag):
        t = pwork.tile([P, NT, WP], F32, tag=tag, name=tag)
        return t, t[:].rearrange("p t w -> p (t w)")

    def wt16(tag, dt=F16):
        t = pw16.tile([P, NT, WP], dt, tag=tag, name=tag)
        return t, t[:].rearrange("p t w -> p (t w)")

    # ---- load input channels (1 DMA per channel, 3D AP) ----
    xa, xaf = wt("A")
    xb, xbf = wt("B")
    xc, xcf = wt("C")
    for c, t in enumerate((xa, xb, xc)):
        nc.sync.dma_start(
            t[:, :, INT],
            xdram[img, c].rearrange("(t p) w -> p t w", p=P))
    yield

    # ---- grayscale (fp32; scale 1/GRAY[2], folded into vertical mats) ----
    nc.vector.scalar_tensor_tensor(xbf[:, PAD:L], xaf[:, PAD:L],
                                   R01, xbf[:, PAD:L], AL.mult, AL.add)
    nc.vector.scalar_tensor_tensor(xcf[:, PAD:L], xbf[:, PAD:L],
                                   R12, xcf[:, PAD:L], AL.mult, AL.add)
    g, gf = xc, xcf          # gray plane, base-2
    nc.scalar.copy(g[:, :, 1:2], g[:, :, 3:4])
    nc.scalar.copy(g[:, :, 0:1], g[:, :, 4:5])
    nc.scalar.copy(g[:, :, WP - 2:WP - 1], g[:, :, WP - 4:WP - 3])
    nc.scalar.copy(g[:, :, WP - 1:WP], g[:, :, WP - 5:WP - 4])
    yield

    # ---- horizontal 5-tap gaussian (fp32), gh base-2 = blur/C0 ----
    a1, a1f = xa, xaf        # xa dead after gray STT1
    a2, a2f = xb, xbf        # xb dead after gray STT2
    nc.gpsimd.tensor_tensor(a1f[:, 0:LV], gf[:, 1:1 + LV], gf[:, 3:3 + LV],
                            AL.add)
    nc.gpsimd.tensor_tensor(a2f[:, 0:LV], gf[:, 0:LV], gf[:, 4:4 + LV],
                            AL.add)
    nc.vector.scalar_tensor_tensor(a1f[:, 0:LV], a2f[:, 0:LV], C2 / C1,
                                   a1f[:, 0:LV], AL.mult, AL.add)
    gh, ghf = wt("D")
    nc.vector.scalar_tensor_tensor(ghf[:, 2:2 + LV], a1f[:, 0:LV], C1 / C0,
                                   gf[:, 2:2 + LV], AL.mult, AL.add)
    yield

    # ---- vertical composite convs on PE (fp32) ----
    # corner-halo chain first so it overlaps the main banded matmuls
    u1, u1f = xa, xaf        # a1 dead after gh
    u2, u2f = xb, xbf        # a2 dead after gh
    cs = psmall.tile([36, W], F32, tag="cs", name="cs")
    for b in range(3):
        nc.sync.dma_start(cs[12 * b:12 * b + 6, :], gh[122:128, b, INT])
        nc.sync.dma_start(cs[12 * b + 6:12 * b + 12, :], gh[0:6, b + 1, INT])
    cos = []
    for ci in (0, 1):
        cps = ppsumc.tile([18, W], F32, tag="cps", name="cps")
        nc.tensor.matmul(cps[:], c_vcor[:, ci, :], cs[:], start=True, stop=True)
        co = psmall.tile([18, W], F32, tag="co", name="co")
        nc.scalar.copy(co[:], cps[:])
        cos.append(co)
    for (cm, u) in ((c_vs, u1), (c_vd, u2)):
        for h in range(2):
            ps = ppsumv.tile([P, 2, W], F32, tag="psv", name="psv")
            for k in range(2):
                t = 2 * h + k
                nc.tensor.matmul(ps[:, k, :], cm[:, t, :], gh[:, t, INT],
                                 start=True, stop=True)
            nc.scalar.copy(u[:, 2 * h:2 * h + 2, INT], ps[:])
    for ci, u in ((0, u1), (1, u2)):
        co = cos[ci]
        for b in range(3):
            nc.sync.dma_start(u[125:128, b, INT], co[6 * b:6 * b + 3, :])
            nc.sync.dma_start(u[0:3, b + 1, INT], co[6 * b + 3:6 * b + 6, :])
    for u in (u1, u2):
        nc.scalar.copy(u[:, :, 1:2], u[:, :, 2:3])
        nc.scalar.copy(u[:, :, WP - 2:WP - 1], u[:, :, WP - 3:WP - 2])
    yield

    # ---- 3-tap horizontal gradients (fp32, base-0) ----
    gx, gxf = gh, ghf        # gh dead after vertical+corner
    ay, ayf = wt("E")
    nc.vector.tensor_tensor(gxf[:, 0:LV], u1f[:, 3:3 + LV], u1f[:, 1:1 + LV],
                            AL.subtract)
    nc.gpsimd.tensor_tensor(ayf[:, 0:LV], u2f[:, 1:1 + LV], u2f[:, 3:3 + LV],
                            AL.add)
    nc.vector.scalar_tensor_tensor(ayf[:, 0:LV], u2f[:, 2:2 + LV], 2.0,
                                   ayf[:, 0:LV], AL.mult, AL.add)
    gy, gyf = ay, ayf
    yield

    # ---- squares (ACT fp32), s16 fp16 compare field, masks ----
    sx, sxf = u1, u1f        # u1 dead after gx
    sy, syf = u2, u2f        # u2 dead after gy
    nc.scalar.activation(sxf[:, 0:LV], gxf[:, 0:LV], AF.Square, 0.0, C0)
    nc.scalar.activation(syf[:, 0:LV], gyf[:, 0:LV], AF.Square, 0.0, C0)
    s16, s16f = wt16("S16")
    nc.vector.tensor_tensor(s16f[:, 2:2 + LV], sxf[:, 0:LV], syf[:, 0:LV],
                            AL.add)
    nc.gpsimd.memset(s16[:, :, 0:PAD], 0.0)
    nc.gpsimd.memset(s16[:, :, WP - PAD:WP], 0.0)
    # masks: notch = (T2*sx < sy), cv = (TH2*sx < sy)  [strict compares only]
    notch, notchf = wt16("NCH", U16)
    cvm, cvmf = wt16("CV", U16)
    nc.vector.scalar_tensor_tensor(notchf[:, 0:LV], sxf[:, 0:LV], T2,
                                   syf[:, 0:LV], AL.mult, AL.is_lt)
    nc.vector.scalar_tensor_tensor(cvmf[:, 0:LV], sxf[:, 0:LV], TH2,
                                   syf[:, 0:LV], AL.mult, AL.is_lt)
    # md1 = (gx*gy > 0): fp16 product then 4x-mode fp16 TS compare
    pxy, pxyf = wt16("PXY")
    nc.vector.tensor_tensor(pxyf[:, 0:LV], gxf[:, 0:LV], gyf[:, 0:LV],
                            AL.mult)
    md1, md1f = wt16("MD", U16)
    nc.vector.tensor_scalar(md1f[:, 0:LV], pxyf[:, 0:LV], 0.0, None, AL.is_gt)
    yield

    # ---- row-shifted planes U16[r]=s16[r+1], D16[r]=s16[r-1] (fp16 PE) ----
    U16t, U16f = wt16("U16")
    D16t, D16f = wt16("D16")
    for (ci, pl) in ((0, U16t), (1, D16t)):
        for h in range(2):
            ps = ppsumv.tile([P, 2, W], F32, tag="psv", name="psv")
            for k in range(2):
                t = 2 * h + k
                nc.tensor.matmul(ps[:, k, :], c_shm16[:, ci, :],
                                 s16[:, t, INT], start=True, stop=True)
            nc.scalar.copy(pl[:, 2 * h:2 * h + 2, INT], ps[:])
    for pl in (U16t, D16t):
        nc.gpsimd.memset(pl[:, :, 0:PAD], 0.0)
        nc.gpsimd.memset(pl[:, :, WP - PAD:WP], 0.0)
    # inter-block boundary rows (one consolidated DMA each) + edge zeros
    nc.sync.dma_start(
        U16t[127:128, 0:NT - 1, INT],
        s16[0:1, 1:NT, INT])
    nc.sync.dma_start(U16t[127:128, NT - 1, INT], zrow16[0:1, :])
    nc.sync.dma_start(
        D16t[0:1, 1:NT, INT],
        s16[127:128, 0:NT - 1, INT])
    nc.gpsimd.memset(D16t[0:1, 0, INT], 0.0)
    yield

    # ---- pair maxes + select chain (fp16) ----
    selx, selxf = wt16("SELX")   # base m3, becomes diag/vert select
    m1t, m1tf = wt16("M1")
    mvt, mvtf = wt16("MV")
    sel, self_ = wt16("SEL")     # base mh, becomes final selection
    nc.vector.tensor_tensor(selxf[:, 0:LV], U16f[:, 1:1 + LV],
                            D16f[:, 3:3 + LV], AL.max)
    nc.vector.tensor_tensor(m1tf[:, 0:LV], U16f[:, 3:3 + LV],
                            D16f[:, 1:1 + LV], AL.max)
    nc.vector.tensor_tensor(mvtf[:, 0:LV], U16f[:, 2:2 + LV],
                            D16f[:, 2:2 + LV], AL.max)
    nc.vector.tensor_tensor(self_[:, 0:LV], s16f[:, 1:1 + LV],
                            s16f[:, 3:3 + LV], AL.max)
    nc.vector.copy_predicated(selxf[:, 0:LV], md1f[:, 0:LV], m1tf[:, 0:LV])
    nc.vector.copy_predicated(selxf[:, 0:LV], cvmf[:, 0:LV], mvtf[:, 0:LV])
    nc.vector.copy_predicated(self_[:, 0:LV], notchf[:, 0:LV], selxf[:, 0:LV])
    yield

    # ---- keep, magnitude, clip, store (fp16 out) ----
    keep, keepf = m1t, m1tf      # m1 consumed by first cp
    nc.vector.tensor_tensor(keepf[:, 0:LV], s16f[:, 2:2 + LV],
                            self_[:, 0:LV], AL.is_gt)
    mag, magf = mvt, mvtf        # mv consumed by second cp
    nc.scalar.activation(magf[:, 0:LV], s16f[:, 2:2 + LV], AF.Sqrt,
                         0.0, 1.0)
    nc.vector.tensor_scalar(magf[:, 0:LV], magf[:, 0:LV], 1.0, None, AL.min)
    out16, out16f = selx, selxf  # selx consumed by final cp
    nc.vector.tensor_tensor(out16f[:, 0:LV], magf[:, 0:LV], keepf[:, 0:LV],
                            AL.mult)
    nc.sync.dma_start(
        ydram[img].rearrange("(t p) w -> p t w", p=P),
        out16[:, :, 0:W])
    yield


def _build():
    import concourse.bacc as bacc
    import concourse.mybir as mybir
    from concourse import tile
    F32 = mybir.dt.float32
    F16 = mybir.dt.float16

    nc = bacc.Bacc("TRN2", target_bir_lowering=False, debug=False,
                   num_devices=NCORES)
    xdram = nc.declare_dram_parameter("xc", [NI, 3, H, W], F32, isOutput=False)
    c_vs_d = nc.declare_dram_parameter("vs", [P, NT, P], F32, isOutput=False)
    c_vd_d = nc.declare_dram_parameter("vd", [P, NT, P], F32, isOutput=False)
    c_vcor_d = nc.declare_dram_parameter("vcor", [36, 2, 18], F32,
                                         isOutput=False)
    c_shm_d = nc.declare_dram_parameter("shm16", [P, 2, P], F16,
                                        isOutput=False)
    ydram = nc.declare_dram_parameter("y", [NI, H, W], F16, isOutput=True)

    with tile.TileContext(nc) as tc:
        with tc.tile_pool(name="pconst", bufs=1) as pconst, \
             tc.tile_pool(name="pwork", bufs=2) as pwork, \
             tc.tile_pool(name="pw16", bufs=2) as pw16, \
             tc.tile_pool(name="psmall", bufs=2) as psmall, \
             tc.tile_pool(name="ppsumv", bufs=3, space="PSUM") as ppsumv, \
             tc.tile_pool(name="ppsumc", bufs=2, space="PSUM") as ppsumc:
            c_vs = pconst.tile([P, NT, P], F32, tag="cvs")
            nc.sync.dma_start(c_vs[:], c_vs_d[:])
            c_vd = pconst.tile([P, NT, P], F32, tag="cvd")
            nc.sync.dma_start(c_vd[:], c_vd_d[:])
            c_vcor = pconst.tile([36, 2, 18], F32, tag="cvcor")
            nc.sync.dma_start(c_vcor[:], c_vcor_d[:])
            c_shm16 = pconst.tile([P, 2, P], F16, tag="cshm")
            nc.sync.dma_start(c_shm16[:], c_shm_d[:])
            zrow16 = pconst.tile([1, W], F16, tag="zr16")
            nc.gpsimd.memset(zrow16[:], 0.0)

            pools = (pwork, pw16, psmall, (ppsumv, ppsumc))
            tens = (xdram, ydram, c_vs, c_vd, c_vcor, c_shm16, zrow16)
            import os
            nrep = int(os.environ.get("KREPEAT", "1"))
            for rep in range(nrep):
                gens = [_emit_image(nc, tc, pools, tens, img)
                        for img in range(NI)]
                done = [False] * NI
                while not all(done):
                    for i, gi in enumerate(gens):
                        if not done[i]:
                            try:
                                next(gi)
                            except StopIteration:
                                done[i] = True

    nc.compile()
    return nc


def _get_nc():
    if "nc" not in _CACHE:
        _CACHE["nc"] = _build()
        _CACHE["consts"] = _build_consts()
    return _CACHE["nc"], _CACHE["consts"]


def kernel(x):
    from concourse.bass_utils import run_bass_kernel_spmd
    x = np.ascontiguousarray(np.asarray(x), dtype=np.float32)
    assert x.shape == (16, 3, H, W), x.shape
    nc, consts = _get_nc()
    in_maps = []
    for c in range(NCORES):
        m = {"xc": x[NI * c:NI * (c + 1)]}
        m.update(consts)
        in_maps.append(m)
    res = run_bass_kernel_spmd(nc, in_maps, list(range(NCORES)))
    y = np.concatenate([res.results[c]["y"] for c in range(NCORES)], axis=0)
    y = y.astype(np.float32)[:, None]          # widen fp16 -> f32, add C dim
    return np.repeat(y, 3, axis=1)             # replicate 3 identical channels


if __name__ == "__main__":
    import golden
    rng = np.random.default_rng(0)
    x = rng.random((16, 3, H, W), dtype=np.float32)
    y = kernel(x)
    ref = golden.reference_np(x)
    d = y - ref
    print("L2 rel:", np.linalg.norm(d) / np.linalg.norm(ref))
    print("absmax:", np.abs(d).max(), " bigpix:", (np.abs(d) > 1e-3).sum())
